# revision 21
# baseline (speedup 1.0000x reference)
"""EventPillarsScatter Trainium2 kernel.

Scatters N=120000 pillar feature vectors (64 f32 channels each) into a dense
BEV canvas [1, 64, 512, 512] at unique flat (y, x) cells.

Strategy (8 NeuronCores, full I/O):
- Host: flat cell idx = y*512 + x. Shard the *canvas columns* 8 ways: core k
  owns columns [k*32768, (k+1)*32768). Pillars are routed to the core owning
  their column, sorted by column, and packed into a per-core feature table
  (rows = pillars in column order, plus a small pool of zero rows at the end
  used as the gather source for empty columns). An int16 inverse-permutation
  table maps each of the core's 32768 columns to its source row.
- Device (per core, identical program = SPMD):
    1. dma_gather (SWDGE): for each output column, fetch its 256B source row
       from DRAM into SBUF, 4096 columns per gather, token i -> partition
       i%128 -> tile [128, 32, 64] (column-major groups of 128 columns).
    2. PE transpose: host-paired adjacent groups -> one [128, 128]
       transposed tile in PSUM ([ch | ch+64] x column), via identity matmul;
       16 fills of [128, 1024] across 4 double-buffered 2-bank PSUM tensors.
    3. ACT/DVE alternate draining PSUM fills into an SBUF canvas
       [128 partitions, 16384] (partitions 0-63 = channels of columns
       [0,16384), partitions 64-127 = channels of columns [16384,32768)).
    4. HWDGE writeout: 8 chunked DMAs SBUF canvas -> DRAM out [2, 64, 16384].
- Host: concatenate the 8 core canvases -> [1, 64, 512, 512].

Self-contained: only needs numpy + the concourse/bass runtime available in
this container.
"""

import numpy as np

import concourse.bacc as bacc
import concourse.mybir as mybir
from concourse.bass_utils import run_bass_kernel_spmd
from concourse.library_config import mlp

# Problem constants (hardcoded per contract).
NY, NX, C, N = 512, 512, 64, 120000
NCORES = 8
COLS = NY * NX                      # 262144
CORE_COLS = COLS // NCORES          # 32768
FEATS_ROWS = 16384                  # per-core feature table rows (pow2, int16-safe)
ZPOOL = 64                          # zero rows at the end of the table
ZBASE = FEATS_ROWS - ZPOOL          # 16320; real rows must stay below this
# single_packet gathers concatenate one 16KB packet per DMA lane; HW caps the
# packet at ~64 data descriptors, so a single_packet gather must not exceed
# 64*16 = 1024 idxs. Larger chunks require single_packet=False.
# Measured on HW: 8 multi-packet gathers of 4096 beat 32 single-packet
# gathers of 1024 (~5 us less Q7/instruction overhead) and 4x8192 is much
# worse (ring pressure).
NCHUNK = 8                          # gather instructions per core
CHUNK_IDXS = CORE_COLS // NCHUNK    # columns per gather
SINGLE_PACKET = False
assert not SINGLE_PACKET or CHUNK_IDXS <= 1024
GROUPS = CORE_COLS // 128           # 256 column-groups of 128
PAIRS = GROUPS // 2                 # 128 transposes (pair g with g+128)
FILL_PAIRS = 8                      # pairs per PSUM fill ([128, 1024] = 2 banks)
FILLS = PAIRS // FILL_PAIRS         # 16
NPSUM = 4                           # PSUM fill buffers (NPSUM*FILL_PAIRS*512B = 8 banks)
WRITES = 8                          # writeout DMAs (2 fills each)

F32 = mybir.dt.float32
I16 = mybir.dt.int16

_NC_CACHE = None


def _build_nc(reps=1):
    """Build the single-core Bass program (shared by all 8 cores, SPMD).

    reps > 1 repeats the whole pipeline back-to-back inside one NEFF (used
    only for benchmarking marginal per-iteration device time)."""
    from contextlib import ExitStack

    nc = bacc.Bacc(
        "TRN2", target_bir_lowering=False, debug=False, num_swdge_queues=4
    )

    feats = nc.dram_tensor("feats", [FEATS_ROWS, C], F32, kind="ExternalInput")
    gidx = nc.dram_tensor("gidx", [128, CORE_COLS // 16], I16, kind="ExternalInput")
    ident = nc.dram_tensor("ident", [128, 128], F32, kind="ExternalInput")
    # out[h, ch, w] = canvas value of channel ch at core-local column 16384*h + w
    out_d = nc.dram_tensor("out", [2, C, CORE_COLS // 2], F32, kind="ExternalOutput")

    with ExitStack() as stack:
        ent = stack.enter_context
        block = ent(nc.Block())
        gbuf = ent(nc.sbuf_tensor("gbuf", [128, GROUPS, C], F32))
        canvas = ent(nc.sbuf_tensor("canvas", [128, CORE_COLS // 2], F32))
        idx_sb = ent(nc.sbuf_tensor("idx_sb", [128, CORE_COLS // 16], I16))
        id_sb = ent(nc.sbuf_tensor("id_sb", [128, 128], F32))
        psum = [
            ent(nc.psum_tensor(f"ps{t}", [128, FILL_PAIRS * 128], F32))
            for t in range(NPSUM)
        ]
        io_idx = ent(nc.semaphore("io_idx"))
        io_id = ent(nc.semaphore("io_id"))
        gsem = [ent(nc.semaphore(f"g{c}")) for c in range(NCHUNK)]
        pe_sem = ent(nc.semaphore("pe_sem"))
        act_sem = ent(nc.semaphore("act_sem"))
        dve_sem = ent(nc.semaphore("dve_sem"))
        outd = ent(nc.semaphore("outd"))

        # Flat free-dim view of the gather buffer: [p, GROUPS*C]. The host
        # orders gather tokens so that slots 2t/2t+1 of a chunk hold the
        # lower/upper canvas half columns of the same free offset; a pair is
        # then one contiguous [128, 128] slice (single free dim, as the BIR
        # verifier requires for the matmul stationary operand).
        gflat = gbuf[:, :, :].rearrange("p g c -> p (g c)")

        ppc = CHUNK_IDXS // 256   # pairs per gather chunk
        gpc = CHUNK_IDXS // 128   # dst groups per chunk
        idw = CHUNK_IDXS // 16    # idx-tile columns per chunk
        wq = CORE_COLS // 2 // WRITES  # 4096 columns per writeout

        # Writeouts alternate between the two HWDGE rings (SP via nc.sync and
        # ACT via nc.scalar) so their descriptor streams interleave with the
        # gather queues across the 16 SDMA engines. Odd writeouts are issued
        # from the ACT engine stream, interleaved with its PSUM-drain copies.
        @block.sync
        def _(sync):
            sync.dma_start(idx_sb[:, :], gidx[:, :]).then_inc(io_idx, 16)
            sync.dma_start(id_sb[:, :], ident[:, :]).then_inc(io_id, 16)
            fpw = FILLS // WRITES  # fills per writeout
            for r in range(reps):
                for k in range(0, WRITES, 2):
                    # writeout k needs fills [k*fpw, (k+1)*fpw) of this rep
                    last = (k + 1) * fpw - 1
                    na = FILLS // 2 * r + (last // 2 + 1)       # ACT fills <= last
                    nd = FILLS // 2 * r + ((last - 1) // 2 + 1) # DVE fills <= last
                    if na > FILLS // 2 * r:
                        sync.wait_ge(act_sem, na)
                    if nd > FILLS // 2 * r:
                        sync.wait_ge(dve_sem, nd)
                    sync.dma_start(
                        out_d[:, :, k * wq : (k + 1) * wq],
                        canvas[:, k * wq : (k + 1) * wq],
                    ).then_inc(outd, 16)
            sync.wait_ge(outd, 16 * WRITES * reps)

        @block.gpsimd
        def _(gp):
            gp.load_library(mlp)
            gp.wait_ge(io_idx, 16)  # idx tile resident
            for r in range(reps):
                for c in range(NCHUNK):
                    if r > 0:
                        # chunk c's gbuf region consumed by pairs < ppc*(c+1)
                        gp.wait_ge(pe_sem, PAIRS * (r - 1) + ppc * (c + 1))
                    gp.dma_gather(
                        gbuf[:, gpc * c : gpc * (c + 1), :],
                        feats[:, :],
                        idx_sb[:, idw * c : idw * (c + 1)],
                        CHUNK_IDXS,
                        CHUNK_IDXS,
                        C,
                        queue_num=c % 4,
                        single_packet=SINGLE_PACKET,
                    ).then_inc(gsem[c], 16)

        @block.tensor
        def _(pe):
            pe.wait_ge(io_id, 16)  # identity resident
            for r in range(reps):
                for g in range(PAIRS):
                    f, s = divmod(g, FILL_PAIRS)
                    F = FILLS * r + f  # global fill index
                    if g % ppc == 0:
                        pe.wait_ge(gsem[g // ppc], 16 * (r + 1))
                    # (ppc pairs per chunk; fills are finer than chunks)
                    if s == 0 and F >= NPSUM:
                        # reuse of psum[F%NPSUM]: fill F-NPSUM must be drained
                        if F % 2 == 0:
                            pe.wait_ge(act_sem, (F - NPSUM) // 2 + 1)
                        else:
                            pe.wait_ge(dve_sem, (F - NPSUM - 1) // 2 + 1)
                    nc.tensor.matmul(
                        psum[f % NPSUM][:, s * 128 : (s + 1) * 128],
                        gflat[:, g * 128 : (g + 1) * 128],
                        id_sb[:, :],
                        start=(s % 4 == 0),
                        stop=(s % 4 == 3),
                        is_transpose=True,
                    ).then_inc(pe_sem, 1)

        fcols = FILL_PAIRS * 128  # canvas columns per fill
        fpw = FILLS // WRITES  # fills per writeout (2)

        @block.scalar
        def _(act):
            for r in range(reps):
                for f in range(0, FILLS, 2):
                    act.wait_ge(pe_sem, PAIRS * r + FILL_PAIRS * (f + 1))
                    if r > 0:
                        # canvas region still being read by a rep r-1 writeout
                        k_of_f = f * WRITES // FILLS
                        act.wait_ge(outd, 16 * (WRITES * (r - 1) + k_of_f + 1))
                    nc.scalar.copy(
                        canvas[:, fcols * f : fcols * (f + 1)], psum[f % NPSUM][:, :]
                    ).then_inc(act_sem, 1)
                    # After copying fill f (even), writeout k = f//2 - 1 (odd k
                    # covering fills f-2, f-1) has all its fills drained once
                    # DVE finishes fill f-1: issue it from this (ACT) ring.
                    k = f // 2 - 1
                    if k >= 1 and k % 2 == 1:
                        # fills f-2 (ACT) and f-1 (DVE) must be fully drained;
                        # the ACT-side wait is on this engine's own sem so it
                        # is satisfied immediately, but makes the copy's SBUF
                        # writes visible to the SDMA reads.
                        act.wait_ge(act_sem, FILLS // 2 * r + f // 2)
                        act.wait_ge(dve_sem, FILLS // 2 * r + (f - 1 - 1) // 2 + 1)
                        nc.scalar.dma_start(
                            out_d[:, :, k * wq : (k + 1) * wq],
                            canvas[:, k * wq : (k + 1) * wq],
                        ).then_inc(outd, 16)
                # trailing odd writeout (k = WRITES-1, fills 14, 15)
                k = WRITES - 1
                act.wait_ge(act_sem, FILLS // 2 * (r + 1))
                act.wait_ge(dve_sem, FILLS // 2 * (r + 1))
                nc.scalar.dma_start(
                    out_d[:, :, k * wq : (k + 1) * wq],
                    canvas[:, k * wq : (k + 1) * wq],
                ).then_inc(outd, 16)

        @block.vector
        def _(dve):
            for r in range(reps):
                for f in range(1, FILLS, 2):
                    dve.wait_ge(pe_sem, PAIRS * r + FILL_PAIRS * (f + 1))
                    if r > 0:
                        k_of_f = f * WRITES // FILLS
                        dve.wait_ge(outd, 16 * (WRITES * (r - 1) + k_of_f + 1))
                    nc.vector.tensor_copy(
                        canvas[:, fcols * f : fcols * (f + 1)], psum[f % NPSUM][:, :]
                    ).then_inc(dve_sem, 1)

    nc.compile()
    return nc


def get_nc():
    global _NC_CACHE
    if _NC_CACHE is None:
        _NC_CACHE = _build_nc()
    return _NC_CACHE


def _prep_core_inputs(voxel_features, flat_idx, zmode="end"):
    """Build per-core feats / gidx arrays from full inputs.

    zmode "end": zero pool of 64 rows at the end of the table (empty columns
    rotate through it, spreading reads over HBM channels);
    "end256": 256-row end pool; "periodic": 64-row pools every 4096 slots;
    "embed": one zero row every 256th slot (measured slower: consecutive
    empties hit one address -> HBM channel hotspot)."""
    in_maps = []
    ident = np.eye(128, dtype=np.float32)
    for k in range(NCORES):
        lo = k * CORE_COLS
        mask = (flat_idx >= lo) & (flat_idx < lo + CORE_COLS)
        local_col = flat_idx[mask] - lo
        order = np.argsort(local_col, kind="stable")
        local_col = local_col[order]
        n_k = local_col.shape[0]
        assert n_k <= ZBASE, f"core {k} has {n_k} pillars > {ZBASE}"

        feats = np.zeros((FEATS_ROWS, C), dtype=np.float32)
        cols = np.arange(CORE_COLS, dtype=np.int64)
        if zmode == "embed":
            # real rows fill slots in column order, skipping every 256th slot
            # (s % 256 == 255), which stays zero: empty columns gather from
            # the nearest such zero slot so reads stay within the sequential
            # stream's HBM neighborhood.
            rank = np.arange(n_k, dtype=np.int64)
            slot = rank + rank // 255
            assert slot.max(initial=0) < FEATS_ROWS
            feats[slot] = voxel_features[mask][order]
            running = np.cumsum(np.isin(cols, local_col)) - 1
            near_zero = (np.maximum(running, 0) // 255) * 256 + 255
            inv = np.minimum(near_zero, FEATS_ROWS - 1)
            inv[local_col] = slot
        elif zmode == "end":
            feats[:n_k] = voxel_features[mask][order]
            inv = ZBASE + (cols & (ZPOOL - 1))  # empty -> zero-pool row
            inv[local_col] = np.arange(n_k, dtype=np.int64)
        elif zmode == "end256":
            feats[:n_k] = voxel_features[mask][order]
            inv = (FEATS_ROWS - 256) + (cols & 255)
            assert n_k <= FEATS_ROWS - 256
            inv[local_col] = np.arange(n_k, dtype=np.int64)
        elif zmode == "periodic":
            # 4032 real rows then 64 zero rows, repeating
            rank = np.arange(n_k, dtype=np.int64)
            slot = rank + 64 * (rank // 4032)
            assert slot.max(initial=0) < FEATS_ROWS
            feats[slot] = voxel_features[mask][order]
            running = np.cumsum(np.isin(cols, local_col)) - 1
            region = np.maximum(running, 0) // 4032
            inv = np.minimum(4096 * region + 4032 + (cols & 63), FEATS_ROWS - 1)
            inv[local_col] = slot
        else:
            raise ValueError(zmode)

        # token order: token i of chunk c, slot s (128 tokens each) sources
        # the row for canvas column 16384h + 128g + p with h = s&1 and pair
        # g = 4c + (s>>1): slots 2u/2u+1 hold the lower/upper half columns of
        # one canvas free offset, so a pair is contiguous in the gather buf.
        i = cols
        c, r = np.divmod(i, CHUNK_IDXS)
        s, p = np.divmod(r, 128)
        h, u = s & 1, s >> 1
        g = (CHUNK_IDXS // 256) * c + u
        gidx_flat = inv[16384 * h + 128 * g + p].astype(np.int16)

        # wrap: position i of chunk c -> partition i%16, free col 256*c + i//16;
        # the 16-partition block is replicated to all 8 partition groups (each
        # Q7 core pair reads the idx tile through its own SBUF port window).
        wrapped = np.tile(
            gidx_flat.reshape(NCHUNK, CHUNK_IDXS // 16, 16)
            .transpose(2, 0, 1)
            .reshape(16, CORE_COLS // 16),
            (8, 1),
        )

        in_maps.append({"feats": feats, "gidx": wrapped, "ident": ident})
    return in_maps


def _run(voxel_features, coords, trace=False, **kw):
    voxel_features = np.ascontiguousarray(voxel_features, dtype=np.float32)
    coords = np.asarray(coords)
    flat_idx = coords[:, 1].astype(np.int64) * NX + coords[:, 2].astype(np.int64)
    in_maps = _prep_core_inputs(voxel_features, flat_idx)
    nc = get_nc()
    res = run_bass_kernel_spmd(
        nc, in_maps, core_ids=list(range(NCORES)), trace=trace, **kw
    )
    canvas = np.concatenate(
        [r["out"].transpose(1, 0, 2).reshape(C, CORE_COLS) for r in res.results],
        axis=1,
    )
    return canvas.reshape(1, C, NY, NX), res


def kernel(voxel_features, coords):
    out, _ = _run(voxel_features, coords, trace=False)
    return out
